# revision 1
# baseline (speedup 1.0000x reference)
"""GATNet (2-layer GAT, 50000 nodes / 800000 edges) on 8 Trainium2 cores.

Strategy: dst-sharding. Each core owns N/8 destination nodes; all 850K edges
(incl. self-loops) are bucketed by dst owner on the host. Attention softmax
denominators and message aggregation are per-dst segment sums, so they are
fully local to a core; the only collective is a tiny AllGather of the layer-2
per-node records (h2@W2 | attention scalars), 1.6MB/core.

Per-core pipeline (single uniform SPMD program; all per-core variation is in
data):
  N1: R1[n] = [x@W1 | x@(W1@A_src)] node table in DRAM (gather source)
  N2: al_dst1 for owned (relabeled) nodes -> SBUF
  E1: per 128-dst block: dma_gather R1[src] (1280B rows, lo/hi int16 split),
      attention logits via S^T-matmul expansion of al_dst, exp on ACT,
      per-edge weighting on DVE, segment-sum via S-matmul into PSUM
      (denominator rides as 8 extra matmul columns), finalize = div + ELU,
      then transpose + matmul to build layer-2 records, unpermute via a
      permutation matmul, write REC.
  AG: AllGather REC -> R2 [N, 64]
  E2: same edge pipeline on 64B records, log_softmax, unpermute, write OUT.

Edges are tiled into 128-edge tiles confined to one 32-row dst quarter of
their block (psum write offsets must be 32-aligned); dst labels are permuted
within each block to balance quarter loads. Tile counts per quarter/kind are
global maxima so the instruction stream is identical on every core.
"""

import sys
import numpy as np

sys.path.insert(0, "/opt/trn_rl_repo")

NCORES = 8
BLK = 128
WIN = 32
NQ = 4
TILE = 128
LO_LIM = 32768
HEADS, HID, OUT_CH = 8, 32, 16
F1 = HEADS * HID            # 256
REC_W = 64                  # layer-2 record row (16 + 1 + 1 padded to 256B)
R1_W = 320                  # layer-1 record row (256 + 8 padded to 1280B)
NEG_SLOPE = 0.2
DEN_EPS = 1e-30


# ---------------------------------------------------------------- planning

def _relabel_blocks(s_k, d_k, n_real, nblk, lo_lim):
    lo_cnt = np.bincount(d_k[s_k < lo_lim], minlength=nblk * BLK)
    hi_cnt = np.bincount(d_k[s_k >= lo_lim], minlength=nblk * BLK)
    new_of_old = np.empty(nblk * BLK, dtype=np.int64)
    for b in range(nblk):
        lo = b * BLK
        hi_b = min((b + 1) * BLK, n_real)
        nreal = hi_b - lo
        new_of_old[lo:lo + BLK] = np.arange(lo, lo + BLK)
        if nreal <= 0:
            continue
        deg_lo = lo_cnt[lo:lo + nreal]
        deg_hi = hi_cnt[lo:lo + nreal]
        order = np.argsort(-(deg_lo + deg_hi), kind="stable")
        qcap = np.array([min(WIN, max(0, nreal - WIN * q)) for q in range(NQ)])
        qlo = np.zeros(NQ, np.int64)
        qhi = np.zeros(NQ, np.int64)
        qn = np.zeros(NQ, np.int64)
        qpos = [[] for _ in range(NQ)]
        for i in order:
            best, bestkey = -1, None
            for q in range(NQ):
                if qn[q] >= qcap[q]:
                    continue
                key = (max(qlo[q] + deg_lo[i], (qhi[q] + deg_hi[i]) * 1.9),
                       qlo[q] + qhi[q])
                if best < 0 or key < bestkey:
                    best, bestkey = q, key
            qpos[best].append(i)
            qlo[best] += deg_lo[i]
            qhi[best] += deg_hi[i]
            qn[best] += 1
        for q in range(NQ):
            base = lo + WIN * q
            for j, i in enumerate(qpos[q]):
                new_of_old[lo + i] = base + j
    old_of_new = np.empty_like(new_of_old)
    old_of_new[new_of_old] = np.arange(nblk * BLK)
    return new_of_old, old_of_new


class _P:
    pass


def _plan(edge_index, n_nodes, lo_lim=LO_LIM):
    ndst = n_nodes // NCORES
    nblk = (ndst + BLK - 1) // BLK
    src = np.concatenate([edge_index[0], np.arange(n_nodes)]).astype(np.int64)
    dst = np.concatenate([edge_index[1], np.arange(n_nodes)]).astype(np.int64)
    owner = dst // ndst
    per_core = []
    T_LO_Q = T_HI_Q = 0
    for k in range(NCORES):
        m = owner == k
        s_k = src[m]
        d_k = dst[m] - k * ndst
        new_of_old, old_of_new = _relabel_blocks(s_k, d_k, ndst, nblk, lo_lim)
        d_rel = new_of_old[d_k]
        per_core.append((s_k, d_rel, new_of_old, old_of_new))
        blkq = d_rel // WIN
        is_lo = s_k < lo_lim
        clo = np.bincount(blkq[is_lo], minlength=nblk * NQ)
        chi = np.bincount(blkq[~is_lo], minlength=nblk * NQ)
        T_LO_Q = max(T_LO_Q, int(-(-clo.max() // TILE)))
        T_HI_Q = max(T_HI_Q, int(-(-chi.max() // TILE)))
    pl = _P()
    pl.ndst, pl.nblk = ndst, nblk
    pl.lo_lim = lo_lim
    pl.n_nodes = n_nodes
    pl.T_LO_Q, pl.T_HI_Q = T_LO_Q, T_HI_Q
    pl.T_LO, pl.T_HI = T_LO_Q * NQ, T_HI_Q * NQ
    pl.T_ALL = pl.T_LO + pl.T_HI
    pl.qot = np.concatenate([np.repeat(np.arange(NQ), T_LO_Q),
                             np.repeat(np.arange(NQ), T_HI_Q)])
    pl.cores = []
    total = nblk * pl.T_ALL * TILE
    pl.total_slots = total
    for k in range(NCORES):
        s_k, d_rel, new_of_old, old_of_new = per_core[k]
        cp = _P()
        cp.new_of_old, cp.old_of_new = new_of_old, old_of_new
        slot_src = np.zeros(total, dtype=np.int64)
        slot_j = np.zeros(total, dtype=np.int64)
        slot_valid = np.zeros(total, dtype=bool)
        is_lo = s_k < lo_lim
        for kind_lo in (True, False):
            sel = np.nonzero(is_lo == kind_lo)[0]
            sel = sel[np.argsort(d_rel[sel], kind="stable")]
            bq = d_rel[sel] // WIN
            bounds = np.searchsorted(bq, np.arange(nblk * NQ + 1))
            ntl = T_LO_Q if kind_lo else T_HI_Q
            for g in range(nblk * NQ):
                lo_i, hi_i = bounds[g], bounds[g + 1]
                if hi_i == lo_i:
                    continue
                b, q = divmod(g, NQ)
                toff = q * ntl if kind_lo else pl.T_LO + q * ntl
                es = sel[lo_i:hi_i]
                for ti in range((len(es) + TILE - 1) // TILE):
                    chunk = es[ti * TILE:(ti + 1) * TILE]
                    base = (b * pl.T_ALL + toff + ti) * TILE
                    n = len(chunk)
                    slot_src[base:base + n] = s_k[chunk]
                    slot_j[base:base + n] = d_rel[chunk] - b * BLK
                    slot_valid[base:base + n] = True
        # pads
        qot_rep = np.tile(np.repeat(pl.qot, TILE), nblk)
        kind_rep = np.tile(np.repeat(np.arange(pl.T_ALL) < pl.T_LO, TILE), nblk)
        pad = ~slot_valid
        slot_j[pad] = (qot_rep * WIN)[pad]
        slot_src[pad] = np.where(kind_rep[pad], 0, lo_lim)
        cp.slot_src, cp.slot_j, cp.slot_valid = slot_src, slot_j, slot_valid
        pl.cores.append(cp)
    return pl


def _build_streams(pl, k):
    """Per-core int16 idx streams, S / S^T selector streams, permutation."""
    cp = pl.cores[k]
    nblk, T_ALL, T_LO, T_HI = pl.nblk, pl.T_ALL, pl.T_LO, pl.T_HI
    NI_LO, NI_HI = T_LO * TILE, T_HI * TILE
    NST = pl.T_LO_Q + pl.T_HI_Q
    src = cp.slot_src.reshape(nblk, T_ALL, TILE)
    jj = cp.slot_j.reshape(nblk, T_ALL, TILE)
    val = cp.slot_valid.reshape(nblk, T_ALL, TILE)
    # idx streams (wrapped [16, NI/16]: slot i -> [i%16, i//16])
    idxl = np.zeros((nblk, 128, max(NI_LO // 16, 1)), np.int16)
    idxh = np.zeros((nblk, 128, max(NI_HI // 16, 1)), np.int16)
    for b in range(nblk):
        # wrapped [16, NI/16]; replicated to all 8 GPSIMD Q7-core groups
        if T_LO:
            v = src[b, :T_LO].reshape(NI_LO)
            idxl[b] = np.tile(v.reshape(NI_LO // 16, 16).T, (8, 1))
        if T_HI:
            v = src[b, T_LO:].reshape(NI_HI) - pl.lo_lim
            idxh[b] = np.tile(v.reshape(NI_HI // 16, 16).T, (8, 1))
    # S stream [nblk, 128, T_ALL, 32]
    S = np.zeros((nblk, 128, T_ALL, WIN), np.float32)
    w = (pl.qot * WIN)[None, :, None]
    jloc = jj - w
    bidx, tidx, pidx = np.nonzero(val)
    S[bidx, pidx, tidx, jloc[val]] = 1.0
    # S^T stream [nblk, 128, NST, 128]: tile ti at partitions 32*q(ti)
    # (must match the expansion rhs partition base), col = rank within quarter
    ST = np.zeros((nblk, 128, NST, 128), np.float32)
    for ti in range(T_ALL):
        q = int(pl.qot[ti])
        r = (ti - q * pl.T_LO_Q) if ti < pl.T_LO \
            else pl.T_LO_Q + (ti - pl.T_LO - q * pl.T_HI_Q)
        ST[:, 32 * q:32 * q + 32, r, :] += S[:, :, ti, :].transpose(0, 2, 1)
    # permutation lhsT [nblk, 128, 128]: PT[j, i] = 1 iff relabeled(i) == j
    PT = np.zeros((nblk, 128, 128), np.float32)
    for b in range(nblk):
        i = np.arange(BLK)
        jrel = cp.new_of_old[b * BLK + i] - b * BLK
        PT[b, jrel, i] = 1.0
    return idxl, idxh, S.reshape(nblk, 128, T_ALL * WIN), \
        ST.reshape(nblk, 128, NST * 128), PT


# ---------------------------------------------------------------- program

def build_program(pl, want_debug=False, dbg=False):
    import concourse.bass as bass
    import concourse.bacc as bacc
    import concourse.tile as tile
    import concourse.mybir as mybir

    F32 = mybir.dt.float32
    I16 = mybir.dt.int16
    AF = mybir.ActivationFunctionType
    ALU = mybir.AluOpType

    n_nodes = pl.n_nodes
    ndst, nblk = pl.ndst, pl.nblk
    T_LO, T_HI, T_ALL = pl.T_LO, pl.T_HI, pl.T_ALL
    NI_LO, NI_HI = T_LO * TILE, T_HI * TILE
    NST = pl.T_LO_Q + pl.T_HI_Q
    NPAD = nblk * BLK
    lo_lim = pl.lo_lim
    hi_rows = n_nodes - lo_lim
    qot = pl.qot

    nc = bacc.Bacc("TRN2", target_bir_lowering=False, debug=want_debug,
                   num_devices=NCORES)
    # -------- inputs
    xT = nc.dram_tensor("xT", [128, n_nodes], F32, kind="ExternalInput")
    xoT = nc.dram_tensor("xoT", [128, NPAD], F32, kind="ExternalInput")
    W1e = nc.dram_tensor("W1e", [128, F1 + HEADS], F32, kind="ExternalInput")
    Vd1 = nc.dram_tensor("Vd1", [128, HEADS], F32, kind="ExternalInput")
    WV2 = nc.dram_tensor("WV2", [128, 2, 18], F32, kind="ExternalInput")
    IDENT = nc.dram_tensor("IDENT", [128, 128], F32, kind="ExternalInput")
    B1R = nc.dram_tensor("B1R", [128, F1], F32, kind="ExternalInput")
    B2R = nc.dram_tensor("B2R", [128, OUT_CH], F32, kind="ExternalInput")
    RECB = nc.dram_tensor("RECB", [128, 18], F32, kind="ExternalInput")
    IDXL = nc.dram_tensor("IDXL", [nblk, 128, max(NI_LO // 16, 1)], I16,
                          kind="ExternalInput")
    IDXH = nc.dram_tensor("IDXH", [nblk, 128, max(NI_HI // 16, 1)], I16,
                          kind="ExternalInput")
    SS = nc.dram_tensor("SS", [nblk, 128, T_ALL * WIN], F32,
                        kind="ExternalInput")
    STT = nc.dram_tensor("STT", [nblk, 128, NST * 128], F32,
                         kind="ExternalInput")
    PT = nc.dram_tensor("PT", [nblk, 128, 128], F32, kind="ExternalInput")
    OUT = nc.dram_tensor("OUT", [ndst, OUT_CH], F32, kind="ExternalOutput")
    if dbg:
        DBG_R1 = nc.dram_tensor("DBG_R1", [n_nodes, R1_W], F32,
                                kind="ExternalOutput")
        DBG_REC = nc.dram_tensor("DBG_REC", [ndst, REC_W], F32,
                                 kind="ExternalOutput")
        DBG_AL1 = nc.dram_tensor("DBG_AL1", [128, nblk, HEADS], F32,
                                 kind="ExternalOutput")
        DBG_G1 = nc.dram_tensor("DBG_G1", [128, 24, R1_W], F32,
                                kind="ExternalOutput")
        DBG_PA = nc.dram_tensor("DBG_PA", [nblk, 128, F1 + HEADS], F32,
                                kind="ExternalOutput")

    NCH = -(-n_nodes // 128)          # node chunks
    BIGC = 8                          # chunks per xT load

    with tile.TileContext(nc) as tc:
        with (
            tc.tile_pool(name="dram", bufs=1, space="DRAM") as dpool,
            tc.tile_pool(name="const", bufs=1) as cpool,
            tc.tile_pool(name="persist", bufs=1) as ppool,
            tc.tile_pool(name="node", bufs=3) as npool,
            tc.tile_pool(name="edge", bufs=2) as epool,
            tc.tile_pool(name="small", bufs=3) as spool,
            tc.tile_pool(name="ps_e", bufs=2, space="PSUM") as ps_e,
            tc.tile_pool(name="ps_a", bufs=2, space="PSUM") as ps_a,
            tc.tile_pool(name="ps_m", bufs=2, space="PSUM") as ps_m,
            tc.tile_pool(name="ps_n", bufs=2, space="PSUM") as ps_n,
        ):
            R1 = dpool.tile([n_nodes, R1_W], F32)
            REC = dpool.tile([ndst, REC_W], F32)
            R2 = dpool.tile([n_nodes, REC_W], F32, addr_space="Shared")

            # consts
            cW1e = cpool.tile([128, F1 + HEADS], F32)
            cVd1 = cpool.tile([128, HEADS], F32)
            cWV2 = cpool.tile([128, 2, 18], F32)
            cID = cpool.tile([128, 128], F32)
            cB1 = cpool.tile([128, F1], F32)
            cB2 = cpool.tile([128, OUT_CH], F32)
            cRB = cpool.tile([128, 18], F32)
            nc.sync.dma_start(cW1e[:], W1e[:])
            nc.sync.dma_start(cVd1[:], Vd1[:])
            nc.sync.dma_start(cWV2[:], WV2[:])
            nc.sync.dma_start(cID[:], IDENT[:])
            nc.sync.dma_start(cB1[:], B1R[:])
            nc.sync.dma_start(cB2[:], B2R[:])
            nc.sync.dma_start(cRB[:], RECB[:])

            alD1 = ppool.tile([128, nblk, HEADS], F32)
            alD2 = ppool.tile([128, nblk, 1], F32)

            # ---------------- N1: R1 node table
            # zero-fill pad columns once (keeps sim clean; gather reads them)
            zpad = cpool.tile([128, R1_W - (F1 + HEADS)], F32)
            nc.vector.memset(zpad[:], 0.0)
            nrow_full = n_nodes // 128
            ZCH = 96            # groups per DMA (keep descriptor count < 16K)
            for g0 in range(0, nrow_full, ZCH):
                g1 = min(g0 + ZCH, nrow_full)
                nc.gpsimd.dma_start(
                    R1[g0 * 128:g1 * 128, F1 + HEADS:R1_W]
                    .rearrange("(g p) e -> p g e", p=128),
                    zpad[:].unsqueeze(1).broadcast_to(
                        [128, g1 - g0, R1_W - (F1 + HEADS)]))
            if n_nodes % 128:
                rem = n_nodes % 128
                nc.gpsimd.dma_start(
                    R1[nrow_full * 128:, F1 + HEADS:R1_W]
                    .rearrange("(g p) e -> p g e", p=rem),
                    zpad[0:rem].unsqueeze(1).broadcast_to(
                        [rem, 1, R1_W - (F1 + HEADS)]))

            for c0 in range(0, NCH, BIGC):
                c1 = min(c0 + BIGC, NCH)
                ncols = min(n_nodes - c0 * 128, BIGC * 128)
                xt = npool.tile([128, BIGC * 128], F32, tag="xt")
                nc.sync.dma_start(xt[:, 0:ncols], xT[:, c0 * 128:c0 * 128 + ncols])
                rw = npool.tile([128, BIGC, R1_W], F32, tag="rw")
                for c in range(c0, c1):
                    m = min(128, n_nodes - c * 128)
                    pn = ps_n.tile([128, F1 + HEADS], F32, tag="pn", padded_shape=[128, 512])
                    nc.tensor.matmul(pn[0:m, :], xt[:, (c - c0) * 128:(c - c0) * 128 + m],
                                     cW1e[:], start=True, stop=True)
                    nc.scalar.copy(rw[0:m, c - c0, 0:F1 + HEADS], pn[0:m, :])
                # write R1 rows [c0*128, c1*128)
                nfull = (min(n_nodes, c1 * 128) - c0 * 128) // 128
                if nfull:
                    nc.sync.dma_start(
                        R1[c0 * 128:c0 * 128 + nfull * 128, 0:F1 + HEADS]
                        .rearrange("(g p) e -> p g e", p=128),
                        rw[:, 0:nfull, 0:F1 + HEADS])
                rem = min(n_nodes, c1 * 128) - c0 * 128 - nfull * 128
                if rem:
                    nc.sync.dma_start(
                        R1[c0 * 128 + nfull * 128:min(n_nodes, c1 * 128), 0:F1 + HEADS]
                        .rearrange("(g p) e -> p g e", p=rem),
                        rw[0:rem, nfull:nfull + 1, 0:F1 + HEADS])

            # ---------------- N2: own al_dst1
            xo = ppool.tile([128, NPAD], F32)
            nc.sync.dma_start(xo[:], xoT[:])
            for b in range(nblk):
                po = ps_n.tile([128, HEADS], F32, tag="pn", padded_shape=[128, 512])
                nc.tensor.matmul(po[:], xo[:, b * 128:(b + 1) * 128], cVd1[:],
                                 start=True, stop=True)
                nc.scalar.copy(alD1[:, b, :], po[:])

            # ---------------- shared edge-stage builder
            def edge_stage(layer):
                F = F1 if layer == 1 else OUT_CH
                H = HEADS if layer == 1 else 1
                EW = R1_W if layer == 1 else REC_W
                alD = alD1 if layer == 1 else alD2
                if layer == 1:
                    tbl_lo = R1[0:min(lo_lim, n_nodes), :]
                    tbl_hi = R1[lo_lim:n_nodes, :] if hi_rows > 0 else None
                else:
                    tbl_lo = R2[0:min(lo_lim, n_nodes), :]
                    tbl_hi = R2[lo_lim:n_nodes, :] if hi_rows > 0 else None
                for b in range(nblk):
                    gl = epool.tile([128, max(T_LO, 1), EW], F32, tag=f"gl{layer}")
                    gh = epool.tile([128, max(T_HI, 1), EW], F32, tag=f"gh{layer}")
                    GCAP = 8          # <=1024 idxs per dma_gather call
                    if T_LO:
                        il = spool.tile([128, NI_LO // 16], I16, tag="il")
                        nc.sync.dma_start(il[:], IDXL[b])
                        for t0 in range(0, T_LO, GCAP):
                            t1 = min(t0 + GCAP, T_LO)
                            ni = (t1 - t0) * TILE
                            nc.gpsimd.dma_gather(
                                gl[:, t0:t1, :], tbl_lo,
                                il[:, t0 * 8:t1 * 8], ni, ni, EW)
                    if T_HI:
                        ih = spool.tile([128, NI_HI // 16], I16, tag="ih")
                        nc.sync.dma_start(ih[:], IDXH[b])
                        for t0 in range(0, T_HI, GCAP):
                            t1 = min(t0 + GCAP, T_HI)
                            ni = (t1 - t0) * TILE
                            nc.gpsimd.dma_gather(
                                gh[:, t0:t1, :], tbl_hi,
                                ih[:, t0 * 8:t1 * 8], ni, ni, EW)
                    sS = spool.tile([128, T_ALL, WIN], F32, tag="sS")
                    nc.sync.dma_start(sS[:], SS[b])
                    sT = spool.tile([128, NST, 128], F32, tag="sT")
                    nc.sync.dma_start(sT[:], STT[b])

                    def gview(ti, sl):
                        t = gl if ti < T_LO else gh
                        i = ti if ti < T_LO else ti - T_LO
                        return t[:, i, sl]

                    # expansion: e_dst per slot
                    pe = ps_e.tile([128, T_ALL * H], F32, tag="pe", padded_shape=[128, 512])
                    for ti in range(T_ALL):
                        q = int(qot[ti])
                        r = (ti - q * pl.T_LO_Q) if ti < T_LO \
                            else pl.T_LO_Q + (ti - T_LO - q * pl.T_HI_Q)
                        nc.tensor.matmul(
                            pe[:, ti * H:(ti + 1) * H],
                            sT[32 * q:32 * q + 32, r, :],
                            alD[32 * q:32 * q + 32, b, :],
                            start=True, stop=True,
                            tile_position=(32 * q, 0))
                    # e = al_src + e_dst ; lrelu ; exp    (in cols F:F+H)
                    for t, tn, off in ((gl, T_LO, 0), (gh, T_HI, T_LO * H)):
                        if not tn:
                            continue
                        ecols = t[:, :, F:F + H]
                        nc.vector.tensor_add(
                            ecols, ecols,
                            pe[:, off:off + tn * H].rearrange(
                                "p (t h) -> p t h", h=H))
                        nc.vector.scalar_tensor_tensor(
                            ecols, ecols, NEG_SLOPE, ecols,
                            op0=ALU.mult, op1=ALU.max)
                        nc.scalar.activation(ecols, ecols, AF.Exp)
                        # weight messages: cols 0:F *= bcast(ex over C)
                        C = F // H
                        nc.vector.tensor_mul(
                            t[:, :, 0:F].rearrange("p t (h c) -> p t h c", c=C),
                            t[:, :, 0:F].rearrange("p t (h c) -> p t h c", c=C),
                            t[:, :, F:F + H].unsqueeze(3).broadcast_to(
                                [128, tn, H, C]))
                    # aggregation into psum [128, F+H] per quarter
                    pa = ps_a.tile([128, F + H], F32, tag="pa", padded_shape=[128, 512])
                    tiles_of_q = [[] for _ in range(NQ)]
                    for ti in range(T_ALL):
                        tiles_of_q[int(qot[ti])].append(ti)
                    for q in range(NQ):
                        tl = tiles_of_q[q]
                        for i, ti in enumerate(tl):
                            nc.tensor.matmul(
                                pa[32 * q:32 * q + 32, :],
                                sS[:, ti, :], gview(ti, slice(0, F + H)),
                                start=(i == 0), stop=(i == len(tl) - 1),
                                tile_position=(0, 32 * q))
                    if dbg and layer == 1 and b == 0:
                        if T_LO:
                            nc.sync.dma_start(DBG_G1[:, 0:T_LO, :], gl[:])
                        if T_HI:
                            nc.sync.dma_start(DBG_G1[:, 16:16 + T_HI, :], gh[:])
                        pacp = spool.tile([128, F + H], F32, tag="pacp")
                        nc.scalar.copy(pacp[:], pa[:])
                        nc.sync.dma_start(DBG_PA[b], pacp[:])
                    # finalize: den -> recip; out = psum * rden (+bias)
                    den = spool.tile([128, H], F32, tag="den")
                    nc.scalar.activation(den[:], pa[:, F:F + H], AF.Copy,
                                         bias=DEN_EPS)
                    rden = spool.tile([128, H], F32, tag="rden")
                    nc.vector.reciprocal(rden[:], den[:])
                    if layer == 1:
                        C = F // H
                        h2t = spool.tile([128, F], F32, tag="h2t")
                        nc.vector.tensor_mul(
                            h2t[:].rearrange("p (h c) -> p h c", c=C),
                            pa[:, 0:F].rearrange("p (h c) -> p h c", c=C),
                            rden[:].unsqueeze(2).broadcast_to([128, H, C]))
                        nc.vector.tensor_add(h2t[:], h2t[:], cB1[:])
                        # ELU without the -1 (folded into record bias):
                        t1 = spool.tile([128, F], F32, tag="t1")
                        nc.vector.tensor_scalar_min(t1[:], h2t[:], 0.0)
                        nc.scalar.activation(t1[:], t1[:], AF.Exp)
                        nc.vector.tensor_scalar_max(h2t[:], h2t[:], 0.0)
                        nc.vector.tensor_add(h2t[:], h2t[:], t1[:])
                        # transpose -> h2T [128c, 2, 128n]
                        ptr = ps_m.tile([128, 2, 128], F32, tag="m", padded_shape=[128, 2, 256])
                        nc.tensor.transpose(ptr[:, 0, :], h2t[:, 0:128], cID[:])
                        nc.tensor.transpose(ptr[:, 1, :], h2t[:, 128:256], cID[:])
                        h2T = spool.tile([128, 2, 128], F32, tag="h2T")
                        nc.scalar.copy(h2T[:], ptr[:])
                        prc = ps_m.tile([128, 18], F32, tag="m", padded_shape=[128, 512])
                        nc.tensor.matmul(prc[:], h2T[:, 0, :], cWV2[:, 0, :],
                                         start=True, stop=False)
                        nc.tensor.matmul(prc[:], h2T[:, 1, :], cWV2[:, 1, :],
                                         start=False, stop=True)
                        rec = spool.tile([128, 18], F32, tag="rec")
                        nc.vector.tensor_add(rec[:], prc[:], cRB[:])
                        # al_d2 own (relabeled order)
                        nc.scalar.copy(alD2[:, b, :], rec[:, 17:18])
                        # unpermute + write REC
                        pp = spool.tile([128, 128], F32, tag="pp")
                        nc.sync.dma_start(pp[:], PT[b])
                        pun = ps_m.tile([128, 18], F32, tag="m", padded_shape=[128, 512])
                        nc.tensor.matmul(pun[:], pp[:], rec[:],
                                         start=True, stop=True)
                        ro = spool.tile([128, 18], F32, tag="ro")
                        nc.scalar.copy(ro[:], pun[:])
                        m = min(BLK, ndst - b * BLK)
                        nc.sync.dma_start(REC[b * BLK:b * BLK + m, 0:18],
                                          ro[0:m, :])
                    else:
                        v = spool.tile([128, OUT_CH], F32, tag="v")
                        nc.vector.tensor_mul(
                            v[:], pa[:, 0:OUT_CH],
                            rden[:].broadcast_to([128, OUT_CH]))
                        nc.vector.tensor_add(v[:], v[:], cB2[:])
                        mx = spool.tile([128, 1], F32, tag="mx")
                        nc.vector.tensor_reduce(mx[:], v[:], op=ALU.max,
                                                axis=mybir.AxisListType.X)
                        nc.vector.tensor_sub(
                            v[:], v[:], mx[:].broadcast_to([128, OUT_CH]))
                        ex = spool.tile([128, OUT_CH], F32, tag="exf")
                        sm = spool.tile([128, 1], F32, tag="sm")
                        nc.scalar.activation(ex[:], v[:], AF.Exp,
                                             accum_out=sm[:])
                        lns = spool.tile([128, 1], F32, tag="lns")
                        nc.scalar.activation(lns[:], sm[:], AF.Ln)
                        nc.vector.tensor_sub(
                            v[:], v[:], lns[:].broadcast_to([128, OUT_CH]))
                        pp = spool.tile([128, 128], F32, tag="pp")
                        nc.sync.dma_start(pp[:], PT[b])
                        pun = ps_m.tile([128, OUT_CH], F32, tag="m", padded_shape=[128, 512])
                        nc.tensor.matmul(pun[:], pp[:], v[:],
                                         start=True, stop=True)
                        vo = spool.tile([128, OUT_CH], F32, tag="vo")
                        nc.scalar.copy(vo[:], pun[:])
                        m = min(BLK, ndst - b * BLK)
                        nc.sync.dma_start(OUT[b * BLK:b * BLK + m, :],
                                          vo[0:m, :])

            if dbg:
                nc.sync.dma_start(DBG_AL1[:], alD1[:])
                nc.sync.dma_start(DBG_R1[:], R1[:])

            # ---------------- E1
            edge_stage(1)
            if dbg:
                nc.sync.dma_start(DBG_REC[:], REC[:])

            # ---------------- zero REC pad cols + AllGather
            z2 = cpool.tile([128, REC_W - 18], F32)
            nc.vector.memset(z2[:], 0.0)
            nfull = ndst // 128
            nc.gpsimd.dma_start(
                REC[0:nfull * 128, 18:REC_W].rearrange("(g p) e -> p g e", p=128),
                z2[:].unsqueeze(1).broadcast_to([128, nfull, REC_W - 18]))
            if ndst % 128:
                rem = ndst % 128
                nc.gpsimd.dma_start(
                    REC[nfull * 128:, 18:REC_W].rearrange("(g p) e -> p g e", p=rem),
                    z2[0:rem].unsqueeze(1).broadcast_to([rem, 1, REC_W - 18]))
            nc.gpsimd.collective_compute(
                "AllGather", mybir.AluOpType.bypass,
                replica_groups=[list(range(NCORES))],
                ins=[REC.opt()], outs=[R2.opt()])

            # ---------------- E2
            edge_stage(2)

    nc.compile()
    return nc


# ---------------------------------------------------------------- host prep

def _host_inputs(pl, inputs):
    x = np.ascontiguousarray(np.asarray(inputs["x"], np.float32))
    W1 = np.asarray(inputs["W1"], np.float32)
    a_s1 = np.asarray(inputs["a_src1"], np.float32)
    a_d1 = np.asarray(inputs["a_dst1"], np.float32)
    b1 = np.asarray(inputs["b1"], np.float32)
    W2 = np.asarray(inputs["W2"], np.float32)
    a_s2 = np.asarray(inputs["a_src2"], np.float32)
    a_d2 = np.asarray(inputs["a_dst2"], np.float32)
    b2 = np.asarray(inputs["b2"], np.float32)
    n_nodes, ndst, nblk = pl.n_nodes, pl.ndst, pl.nblk
    NPAD = nblk * BLK

    A_s1 = np.zeros((F1, HEADS), np.float32)
    A_d1 = np.zeros((F1, HEADS), np.float32)
    for h in range(HEADS):
        A_s1[h * HID:(h + 1) * HID, h] = a_s1[h]
        A_d1[h * HID:(h + 1) * HID, h] = a_d1[h]
    V_s1 = (W1 @ A_s1).astype(np.float32)
    V_d1 = (W1 @ A_d1).astype(np.float32)
    V_s2 = (W2 @ a_s2[0]).astype(np.float32)
    V_d2 = (W2 @ a_d2[0]).astype(np.float32)
    WV2 = np.concatenate([W2, V_s2[:, None], V_d2[:, None]], axis=1)  # [256,18]
    RECB = -WV2.sum(axis=0, keepdims=True)          # [1, 18] (the ELU -1 fold)

    xT = np.ascontiguousarray(x.T)                              # [128, N]
    common = {
        "xT": xT,
        "W1e": np.ascontiguousarray(
            np.concatenate([W1, V_s1], axis=1)),                # [128, 264]
        "Vd1": np.ascontiguousarray(V_d1),                      # [128, 8]
        "WV2": np.ascontiguousarray(
            WV2.reshape(2, 128, 18).transpose(1, 0, 2)),        # [128,2,18]
        "IDENT": np.eye(128, dtype=np.float32),
        "B1R": np.tile(b1[None, :], (128, 1)).astype(np.float32),
        "B2R": np.tile(b2[None, :], (128, 1)).astype(np.float32),
        "RECB": np.tile(RECB, (128, 1)).astype(np.float32),
    }
    in_maps = []
    for k in range(NCORES):
        cp = pl.cores[k]
        idxl, idxh, S, ST, PT = _build_streams(pl, k)
        xo = np.zeros((NPAD, 128), np.float32)
        xo[cp.new_of_old[:ndst]] = x[k * ndst:(k + 1) * ndst]
        m = dict(common)
        m["xoT"] = np.ascontiguousarray(xo.T)
        m["IDXL"] = idxl
        m["IDXH"] = idxh
        m["SS"] = np.ascontiguousarray(S)
        m["STT"] = np.ascontiguousarray(ST)
        m["PT"] = np.ascontiguousarray(PT)
        in_maps.append(m)
    return in_maps


# ---------------------------------------------------------------- entry

_CACHE = {}


def _run(inputs, trace=False, **kw):
    from concourse.bass_utils import run_bass_kernel_spmd

    edge_index = np.asarray(inputs["edge_index"])
    n_nodes = int(np.asarray(inputs["x"]).shape[0])
    pl = _plan(edge_index, n_nodes)
    nc = build_program(pl)
    in_maps = _host_inputs(pl, inputs)
    res = run_bass_kernel_spmd(nc, in_maps, list(range(NCORES)),
                               trace=trace, **kw)
    out = np.concatenate([res.results[k]["OUT"] for k in range(NCORES)], axis=0)
    return out.astype(np.float32), res


def kernel(**inputs):
    out, _ = _run(inputs)
    return out

